# revision 9
# baseline (speedup 1.0000x reference)
"""Channel-attention kernel for Trainium2, data-parallel over batch on 8 NeuronCores.

Reference computation (per batch b):
    xr   = x[b].reshape(HW, C)                  # [4096, 512] fp32
    s    = xr^T @ xr                            # [C, C] gram matrix
    attn = softmax(s, axis=-1)
    v    = xr @ attn                            # [4096, 512]
    out  = beta * v + x[b]

Device strategy (per core: 2 batches):
  - load x[b] fp32 natural layout -> SBUF in [128, 4*512] groups (sync ring)
  - GEMM1 (gram) on TensorE in float32r straight off the fp32 tiles (full rate
    at free dim 512, no cast dependency)
  - SWDGE cast-DMA (gpsimd) bounces each group to a DRAM scratch as bf16;
    per quarter, big DMA-transposes (sync ring) read it back as the
    [C-part, HW-free] bf16 layout for GEMM2's stationary operand
  - softmax rows on DVE+ScalarE straight out of PSUM (ScalarE runs ONLY exp so
    it is never head-of-line blocked); beta is folded into the normalization
    (attn_scaled = beta * exp(s - max) / sum), so the epilogue is one
    PSUM+SBUF add and beta=0 gives bit-exact x
  - GEMM2 on TensorE in bf16 into 2-bank PSUM tiles
  - epilogue on DVE: out = v + x_fp32, store on sync ring

Engine-queue assignment avoids head-of-line blocking: sync = loads +
transposes + output stores (all unblock in lockstep with the compute that
frees their inputs), gpsimd = bounce cast-stores (wait only on their load),
scalar = exp only, vector = reductions + epilogue adds.
"""

import numpy as np

import concourse.bass as bass
import concourse.tile as tile
from concourse import bacc, mybir
from concourse.bass_utils import run_bass_kernel_spmd

N_CORES = 8
B_FULL = 16
B_PER_CORE = B_FULL // N_CORES  # 2
H = 64
W = 64
HW = H * W  # 4096
C = 512
NT = HW // 128  # 32 row tiles
CB = C // 128  # 4 channel blocks
LG = 4  # row tiles per load group
NG = NT // LG  # 8 load groups per batch
NQ = 4  # scratch quarters per batch (2 load groups each)
QROWS = HW // NQ  # 1024 rows per quarter

F32 = mybir.dt.float32
F32R = mybir.dt.float32r
BF16 = mybir.dt.bfloat16
AXL = mybir.AxisListType
ALU = mybir.AluOpType
ACTFN = mybir.ActivationFunctionType


def channel_attention_body(tc, out_ap, x_ap, beta_ap):
    nc = tc.nc
    from contextlib import ExitStack

    with ExitStack() as ctx:
        ep = ctx.enter_context
        xf_pool = ep(tc.tile_pool(name="xf", bufs=10))
        xbf_pool = ep(tc.tile_pool(name="xbf", bufs=9))
        xt_pool = ep(tc.tile_pool(name="xt", bufs=5))
        sm_pool = ep(tc.tile_pool(name="sm", bufs=5))
        st_pool = ep(tc.tile_pool(name="st", bufs=8))
        out_pool = ep(tc.tile_pool(name="outp", bufs=5))
        const_pool = ep(tc.tile_pool(name="const", bufs=1))
        scr_pool = ep(tc.tile_pool(name="scr", bufs=6, space="DRAM"))
        ps_s_pool = ep(tc.tile_pool(name="ps_s", bufs=4, space="PSUM"))
        ps_v_pool = ep(tc.tile_pool(name="ps_v", bufs=2, space="PSUM"))

        # beta -> broadcast to [128, 1]
        beta_sb = const_pool.tile([1, 1], F32, tag="beta")
        nc.sync.dma_start(beta_sb[0:1, 0:1], beta_ap[None, :])
        beta_bc = const_pool.tile([128, 1], F32, tag="beta_bc")
        nc.gpsimd.partition_broadcast(beta_bc[:, :], beta_sb[0:1, :])

        for b in range(B_PER_CORE):
            # ---- load fp32 (1MB groups); SWDGE cast-bounce each group to
            #      DRAM bf16; per quarter DMA-transpose back [C-part, HW] ----
            xf = []
            xbf = []
            xt = [
                xt_pool.tile([128, HW], BF16, tag="xt", name=f"xt_b{b}_{cb}")
                for cb in range(CB)
            ]
            for q in range(NQ):
                scr = scr_pool.tile(
                    [QROWS, C], BF16, tag="scr", name=f"scr_b{b}_q{q}"
                )
                for g2 in range(QROWS // (LG * 128)):  # 2 groups per quarter
                    g = q * 2 + g2
                    r0 = g * LG * 128
                    t = xf_pool.tile([128, LG * C], F32, tag="xf")
                    nc.sync.dma_start(
                        t[:, :].rearrange("p (f c) -> p f c", c=C),
                        x_ap[b, r0 : r0 + LG * 128, :].rearrange(
                            "(f p) c -> p f c", p=128
                        ),
                    )
                    bf = xbf_pool.tile([128, LG * C], BF16, tag="xbf")
                    nc.scalar.copy(bf[:, :], t[:, :])
                    nc.gpsimd.dma_start(
                        scr[g2 * LG * 128 : (g2 + 1) * LG * 128, :].rearrange(
                            "(f p) c -> p f c", p=128
                        ),
                        t[:, :].rearrange("p (f c) -> p f c", c=C),
                    )
                    xf.append(t)
                    xbf.append(bf)
                for cb in range(CB):
                    nc.sync.dma_start(
                        xt[cb][:, q * QROWS : (q + 1) * QROWS],
                        scr[:, cb * 128 : (cb + 1) * 128],
                        transpose=True,
                    )

            # ---- GEMM1 (float32r): s[cb] = sum_nt xf[nt][:, cb]^T @ xf[nt]
            #      nt-major head (lockstep with loads), cb-major tail
            #      (so softmax starts before GEMM1 fully ends) ----
            s_ps = [
                ps_s_pool.tile([128, C], F32, tag="s", name=f"s_b{b}_{cb}")
                for cb in range(CB)
            ]
            TAIL = 8

            def g1mm(nt, cb):
                g, k = divmod(nt, LG)
                nc.tensor.matmul(
                    s_ps[cb][:, :],
                    xbf[g][:, k * C + cb * 128 : k * C + (cb + 1) * 128],
                    xbf[g][:, k * C : (k + 1) * C],
                    start=(nt == 0),
                    stop=(nt == NT - 1),
                )

            for nt in range(NT - TAIL):
                for cb in range(CB):
                    g1mm(nt, cb)
            for cb in range(CB):
                for nt in range(NT - TAIL, NT):
                    g1mm(nt, cb)

            # ---- softmax rows (c on partitions, d on free axis) ----
            attn = []
            for cb in range(CB):
                nmax = st_pool.tile([128, 1], F32, tag="nmax")
                nc.vector.tensor_reduce(
                    nmax[:, :], s_ps[cb][:, :], axis=AXL.X, op=ALU.max, negate=True
                )
                exps = sm_pool.tile([128, C], BF16, tag="exps")
                ssum = st_pool.tile([128, 1], F32, tag="ssum")
                nc.scalar.activation(
                    exps[:, :],
                    s_ps[cb][:, :],
                    ACTFN.Exp,
                    bias=nmax[:, :],
                    scale=1.0,
                    accum_out=ssum[:, :],
                )
                rinv = st_pool.tile([128, 1], F32, tag="rinv")
                nc.vector.reciprocal(rinv[:, :], ssum[:, :])
                rsc = st_pool.tile([128, 1], F32, tag="rsc")
                nc.vector.tensor_mul(rsc[:, :], rinv[:, :], beta_bc[:, :])
                at = sm_pool.tile([128, C], BF16, tag="attn")
                nc.vector.tensor_scalar_mul(at[:, :], exps[:, :], rsc[:, :])
                attn.append(at)

            # ---- GEMM2 (bf16, 2 row tiles per PSUM tile) + epilogue ----
            for np_ in range(NT // 2):
                vps = ps_v_pool.tile([128, 2 * C], F32, tag="v")
                for j in range(2):
                    nt = np_ * 2 + j
                    for cb in range(CB):
                        nc.tensor.matmul(
                            vps[:, j * C : (j + 1) * C],
                            xt[cb][:, nt * 128 : (nt + 1) * 128],
                            attn[cb][:, :],
                            start=(cb == 0),
                            stop=(cb == CB - 1),
                        )
                ot = out_pool.tile([128, 2 * C], F32, tag="o")
                g, kp = divmod(np_, 2)
                nc.vector.tensor_add(
                    ot[:, :], vps[:, :], xf[g][:, kp * 2 * C : (kp + 1) * 2 * C]
                )
                nc.sync.dma_start(
                    out_ap[b, np_ * 256 : (np_ + 1) * 256, :].rearrange(
                        "(f p) c -> p f c", p=128
                    ),
                    ot[:, :].rearrange("p (f c) -> p f c", c=C),
                )


_NC_CACHE = None


def _build():
    global _NC_CACHE
    if _NC_CACHE is not None:
        return _NC_CACHE
    nc = bacc.Bacc(
        "TRN2",
        target_bir_lowering=False,
        debug=False,
        num_devices=N_CORES,
    )
    x_ap = nc.dram_tensor("x", [B_PER_CORE, HW, C], F32, kind="ExternalInput").ap()
    beta_ap = nc.dram_tensor("beta", [1], F32, kind="ExternalInput").ap()
    out_ap = nc.dram_tensor(
        "out", [B_PER_CORE, HW, C], F32, kind="ExternalOutput"
    ).ap()
    with tile.TileContext(nc) as tc:
        channel_attention_body(tc, out_ap, x_ap, beta_ap)
    nc.compile()
    _NC_CACHE = nc
    return nc


def run(x, beta, trace=False, **trace_kwargs):
    """Shard over batch, run on 8 cores, gather. Returns (out, BassKernelResults)."""
    x = np.asarray(x, dtype=np.float32)
    beta = np.asarray(beta, dtype=np.float32)
    assert x.shape == (B_FULL, H, W, C), x.shape
    nc = _build()
    xr = x.reshape(B_FULL, HW, C)
    in_maps = [
        {
            "x": np.ascontiguousarray(
                xr[i * B_PER_CORE : (i + 1) * B_PER_CORE]
            ),
            "beta": beta,
        }
        for i in range(N_CORES)
    ]
    res = run_bass_kernel_spmd(
        nc, in_maps, core_ids=list(range(N_CORES)), trace=trace, **trace_kwargs
    )
    out = np.concatenate([res.results[i]["out"] for i in range(N_CORES)], axis=0)
    return out.reshape(B_FULL, H, W, C), res


def kernel(x, beta):
    out, _ = run(x, beta, trace=False)
    return out
